# revision 7
# baseline (speedup 1.0000x reference)
"""Conditional contrastive loss on 8 TRN2 NeuronCores (Bass/Tile).

Strategy (data-parallel over rows, per sharding hint):
  - Each core owns 512 rows (of 4096) of inst_embed ("x") and proxy ("p").
  - Host passes transposed layouts (matmul-native [D, n]) plus the core's
    pre-gathered positive-selection mask rows (negative_mask[labels]) in bf16.
  - Device normalizes embeddings: squares split across GPSIMD+DVE, partition
    reduction via an all-ones matmul whose M=128 output doubles as a
    partition-broadcast, ACT sqrt + DVE reciprocal (in place), DVE/GPSIMD
    column-scale into fp32r.
  - Similarity rows sim[i, j] for the core's i-block: fp32r matmuls
    accumulated in PSUM over 4 K-chunks, 2048 columns per PSUM group.
  - exp((sim-margin)/T) on the scalar engine straight out of PSUM with
    accum_out = row sums -> denominator for free; z written to SBUF in bf16.
  - numerator = tensor_tensor_reduce(z * mask) on DVE (bf16 2x mode).
  - Device emits ln(den), ln(num) per row for both matrices ([512, 4] f32
    per core); the host does the final O(N) mean + gather across cores.
"""
import numpy as np
import ml_dtypes

import concourse.bacc as bacc
import concourse.tile as tile
from concourse import mybir, bass_utils

N_FULL = 4096
D = 512
C = 100
N_CORES = 8
RP = N_FULL // N_CORES  # rows per core = 512
P = 128                 # SBUF partitions
KC = D // P             # contraction chunks = 4
JT = 512                # columns per PSUM bank
JG = 2048               # columns per PSUM group (4 banks)
NG = N_FULL // JG       # groups per (i-tile, matrix) = 2
IT = RP // P            # i-tiles per core = 4

F32 = mybir.dt.float32
F32R = mybir.dt.float32r
BF16 = mybir.dt.bfloat16
AF = mybir.ActivationFunctionType
ALU = mybir.AluOpType
AX = mybir.AxisListType

_CACHE = {}


def _build(inv_t: float, bias_den: float):
    nc = bacc.Bacc("TRN2", target_bir_lowering=False, debug=False,
                   num_devices=N_CORES)

    xT = nc.dram_tensor("xT", [D, N_FULL], F32, kind="ExternalInput")
    xTc = nc.dram_tensor("xTc", [D, RP], F32, kind="ExternalInput")
    pTc = nc.dram_tensor("pTc", [D, RP], F32, kind="ExternalInput")
    mk = nc.dram_tensor("mk", [RP, N_FULL], BF16, kind="ExternalInput")
    out = nc.dram_tensor("out", [RP, 4], F32, kind="ExternalOutput")

    with tile.TileContext(nc) as tc:
        with (
            tc.tile_pool(name="xpool", bufs=5) as xpool,
            tc.tile_pool(name="big", bufs=1) as big,
            tc.tile_pool(name="binv", bufs=2) as binvp,
            tc.tile_pool(name="zpool", bufs=3) as zpool,
            tc.tile_pool(name="maskp", bufs=1) as maskp,
            tc.tile_pool(name="lhs", bufs=2) as lhs,
            tc.tile_pool(name="small", bufs=1) as small,
            tc.tile_pool(name="ps", bufs=2, space="PSUM") as pspool,
        ):
            # ---- constants ----
            ones_f = small.tile([P, P], F32, name="ones_f")
            nc.vector.memset(ones_f[:], 1.0)
            ones_r = small.tile([P, P], F32R, name="ones_r")
            nc.vector.tensor_copy(ones_r[:], ones_f[:])
            bias_den_t = small.tile([P, 1], F32, name="bias_den_t")
            nc.vector.memset(bias_den_t[:], bias_den)

            # ---- loads (emission order ~ DMA priority) ----
            xtc = []
            ptc = []
            for k in range(KC):
                t = lhs.tile([P, RP], F32, name=f"xtc{k}", tag=f"xtc{k}")
                nc.sync.dma_start(t[:], xTc.ap()[k * P:(k + 1) * P, :])
                xtc.append(t)
                t = lhs.tile([P, RP], F32, name=f"ptc{k}", tag=f"ptc{k}")
                nc.sync.dma_start(t[:], pTc.ap()[k * P:(k + 1) * P, :])
                ptc.append(t)
            xt = []
            for k in range(KC):
                t = xpool.tile([P, N_FULL], F32, name=f"xt{k}", tag="x")
                nc.sync.dma_start(t[:], xT.ap()[k * P:(k + 1) * P, :])
                xt.append(t)
            mask_t = []
            for it in range(IT):
                t = maskp.tile([P, N_FULL], BF16, name=f"mask{it}")
                nc.sync.dma_start(t[:], mk.ap()[it * P:(it + 1) * P, :])
                mask_t.append(t)

            # ---- norms of the core's own x rows and proxy rows (lhsT) ----
            def chunk_norm_inv(src_tiles, label):
                ps_c = pspool.tile([P, JG], F32, name=f"psc_{label}", tag="ps")
                for k in range(KC):
                    sqc = small.tile([P, RP], F32R, name=f"sqc_{label}{k}",
                                     tag="sqc")
                    nc.gpsimd.tensor_tensor(sqc[:], src_tiles[k][:],
                                            src_tiles[k][:], ALU.mult)
                    nc.tensor.matmul(ps_c[:, :RP], ones_r[:], sqc[:],
                                     start=(k == 0), stop=(k == KC - 1))
                inv = small.tile([P, RP], F32, name=f"inv_{label}")
                nc.scalar.activation(inv[:], ps_c[:, :RP], AF.Sqrt)
                nc.vector.reciprocal(inv[:], inv[:])
                return inv

            bx_inv = chunk_norm_inv(xtc, "x")
            bp_inv = chunk_norm_inv(ptc, "p")

            xnc = []
            pnc = []
            for k in range(KC):
                t = lhs.tile([P, RP], F32R, name=f"xnc{k}", tag=f"xtc{k}")
                nc.vector.tensor_tensor(t[:], xtc[k][:], bx_inv[:], ALU.mult)
                xnc.append(t)
                t = lhs.tile([P, RP], F32R, name=f"pnc{k}", tag=f"ptc{k}")
                nc.vector.tensor_tensor(t[:], ptc[k][:], bp_inv[:], ALU.mult)
                pnc.append(t)

            # ---- norms of full x (columns of xT) ----
            # squares split DVE/GPSIMD, per-group norm psums so the main loop
            # can start on group 0 early.
            ps_norm = [
                pspool.tile([P, JG], F32, name=f"ps_norm{g}", tag="ps")
                for g in range(NG)
            ]
            for k in range(KC):
                sq = big.tile([P, N_FULL], F32R, name=f"sq{k}", tag="bigbuf")
                h = N_FULL // 2
                nc.gpsimd.tensor_tensor(sq[:, :h], xt[k][:, :h],
                                        xt[k][:, :h], ALU.mult)
                nc.vector.tensor_tensor(sq[:, h:], xt[k][:, h:],
                                        xt[k][:, h:], ALU.mult)
                for g in range(NG):
                    for jl in range(JG // JT):
                        j0 = g * JG + jl * JT
                        nc.tensor.matmul(
                            ps_norm[g][:, jl * JT:(jl + 1) * JT],
                            ones_r[:],
                            sq[:, j0:j0 + JT],
                            start=(k == 0), stop=(k == KC - 1),
                        )
            # 1/||x_j|| broadcast over partitions (per group)
            b_inv = []
            for g in range(NG):
                t = binvp.tile([P, JG], F32, name=f"binv{g}", tag="binv")
                nc.scalar.activation(t[:], ps_norm[g][:], AF.Sqrt)
                nc.vector.reciprocal(t[:], t[:])
                b_inv.append(t)

            # normalized xT in fp32r (rhs), per (k, g) half for early start;
            # group-0 halves on DVE, group-1 halves on GPSIMD.
            xn = []
            for k in range(KC):
                t = xpool.tile([P, N_FULL], F32R, name=f"xn{k}", tag="x")
                nc.vector.tensor_tensor(t[:, :JG], xt[k][:, :JG],
                                        b_inv[0][:], ALU.mult)
                nc.gpsimd.tensor_tensor(t[:, JG:], xt[k][:, JG:],
                                        b_inv[1][:], ALU.mult)
                xn.append(t)

            # ---- main loop ----
            acc_den = {}
            acc_num = {}
            for it in range(IT):
                for mat in range(2):
                    acc_den[it, mat] = small.tile([P, NG], F32,
                                                  name=f"accd{it}_{mat}")
                    acc_num[it, mat] = small.tile([P, NG], F32,
                                                  name=f"accn{it}_{mat}")

            for it in range(IT):
                i0 = it * P
                for mat in range(2):
                    lh = pnc if mat == 0 else xnc
                    for g in range(NG):
                        ps = pspool.tile([P, JG], F32,
                                         name=f"ps_{it}_{mat}_{g}", tag="ps")
                        for k in range(KC):
                            for jl in range(JG // JT):
                                j0 = g * JG + jl * JT
                                nc.tensor.matmul(
                                    ps[:, jl * JT:(jl + 1) * JT],
                                    lh[k][:, i0:i0 + P],
                                    xn[k][:, j0:j0 + JT],
                                    start=(k == 0), stop=(k == KC - 1),
                                )
                        z = zpool.tile([P, JG], BF16,
                                       name=f"z_{it}_{mat}_{g}", tag="z")
                        nc.scalar.activation(
                            z[:], ps[:], AF.Exp,
                            bias=bias_den_t[:], scale=inv_t,
                            accum_out=acc_den[it, mat][:, g:g + 1],
                        )
                        zo = zpool.tile([P, JG], BF16,
                                        name=f"zo_{it}_{mat}_{g}", tag="zo",
                                        bufs=2)
                        nc.vector.scalar_tensor_tensor(
                            out=zo[:], in0=z[:], scalar=1.0,
                            in1=mask_t[it][:, g * JG:(g + 1) * JG],
                            op0=ALU.mult, op1=ALU.mult,
                            accum_out=acc_num[it, mat][:, g:g + 1],
                        )

            # ---- tail: sum group partials, take logs, store ----
            sums = small.tile([P, 4 * IT], F32, name="sums")
            lns = small.tile([P, 4 * IT], F32, name="lns")
            for it in range(IT):
                for mat in range(2):
                    cd = it * 4 + mat * 2
                    nc.vector.tensor_reduce(sums[:, cd:cd + 1],
                                            acc_den[it, mat][:], AX.X, ALU.add)
                    nc.vector.tensor_reduce(sums[:, cd + 1:cd + 2],
                                            acc_num[it, mat][:], AX.X, ALU.add)
            nc.scalar.activation(lns[:], sums[:], AF.Ln)
            for it in range(IT):
                nc.sync.dma_start(out.ap()[it * P:(it + 1) * P, :],
                                  lns[:, it * 4:(it + 1) * 4])

    nc.compile()
    return nc


def make_in_maps(x, p, nmf, lab):
    xT = np.ascontiguousarray(x.T)
    in_maps = []
    for c in range(N_CORES):
        rows = slice(c * RP, (c + 1) * RP)
        in_maps.append({
            "xT": xT,
            "xTc": np.ascontiguousarray(x[rows].T),
            "pTc": np.ascontiguousarray(p[rows].T),
            "mk": nmf[lab[rows]].astype(ml_dtypes.bfloat16),
        })
    return in_maps


def kernel(inst_embed, proxy, negative_mask, labels, temperature, margin):
    t = float(np.asarray(temperature))
    m = float(np.asarray(margin))
    inv_t = 1.0 / t
    bias_den = -m / t

    key = (t, m)
    if key not in _CACHE:
        _CACHE[key] = _build(inv_t, bias_den)
    nc = _CACHE[key]

    x = np.asarray(inst_embed, dtype=np.float32)
    p = np.asarray(proxy, dtype=np.float32)
    nmf = np.asarray(negative_mask, dtype=np.float32)
    lab = np.asarray(labels).astype(np.int64)

    in_maps = make_in_maps(x, p, nmf, lab)

    res = bass_utils.run_bass_kernel_spmd(nc, in_maps,
                                          core_ids=list(range(N_CORES)))
    outs = np.concatenate([res.results[c]["out"] for c in range(N_CORES)],
                          axis=0)
    ld_p, ln_p, ld_i, ln_i = (outs[:, q].astype(np.float64) for q in range(4))
    loss = (-2.0 * np.log(t)
            + (ld_p - ln_p).mean()
            + (ld_i - ln_i).mean())
    return np.float32(loss)


# revision 8
# speedup vs baseline: 1.5191x; 1.5191x over previous
"""Conditional contrastive loss on 8 TRN2 NeuronCores (Bass/Tile).

Strategy (data-parallel over rows, per sharding hint):
  - Each core owns 512 rows (of 4096) of inst_embed ("x") and proxy ("p").
  - Host passes transposed layouts (matmul-native [D, n]) plus the core's
    pre-gathered positive-selection mask rows (negative_mask[labels]) in bf16.
  - Device normalization: squares on the scalar engine (fp32r out),
    partition-reduction via an all-ones matmul whose M=128 output doubles as
    a partition-broadcast, then 1/sqrt via ACT ln + exp(-0.5*u) (the DVE
    reciprocal is 8 cycles/element - avoid), and DVE/GPSIMD column-scale
    into fp32r.
  - Similarity rows sim[i, j] for the core's i-block: fp32r matmuls
    accumulated in PSUM over 4 K-chunks, 2048 columns per PSUM group.
  - exp((sim-margin)/T) on the scalar engine straight out of PSUM with
    accum_out = row sums -> denominator for free; z written to SBUF in bf16.
  - numerator = scalar_tensor_tensor(z * mask) on DVE with accum_out.
  - Main loop runs group-0 columns first so it can start before group-1
    normalization finishes.
  - Device emits ln(den), ln(num) per row for both matrices ([512, 4] f32
    per core); the host does the final O(N) mean + gather across cores.
"""
import numpy as np
import ml_dtypes

import concourse.bacc as bacc
import concourse.tile as tile
from concourse import mybir, bass_utils

N_FULL = 4096
D = 512
C = 100
N_CORES = 8
RP = N_FULL // N_CORES  # rows per core = 512
P = 128                 # SBUF partitions
KC = D // P             # contraction chunks = 4
JT = 512                # columns per PSUM bank
JG = 2048               # columns per PSUM group (4 banks)
NG = N_FULL // JG       # groups per (i-tile, matrix) = 2
IT = RP // P            # i-tiles per core = 4

F32 = mybir.dt.float32
F32R = mybir.dt.float32r
BF16 = mybir.dt.bfloat16
AF = mybir.ActivationFunctionType
ALU = mybir.AluOpType
AX = mybir.AxisListType

_CACHE = {}


def _build(inv_t: float, bias_den: float):
    nc = bacc.Bacc("TRN2", target_bir_lowering=False, debug=False,
                   num_devices=N_CORES)

    xT = nc.dram_tensor("xT", [D, N_FULL], F32, kind="ExternalInput")
    xTc = nc.dram_tensor("xTc", [D, RP], F32, kind="ExternalInput")
    pTc = nc.dram_tensor("pTc", [D, RP], F32, kind="ExternalInput")
    mk = nc.dram_tensor("mk", [RP, N_FULL], BF16, kind="ExternalInput")
    out = nc.dram_tensor("out", [RP, 4], F32, kind="ExternalOutput")

    with tile.TileContext(nc) as tc:
        with (
            tc.tile_pool(name="xpool", bufs=5) as xpool,
            tc.tile_pool(name="big", bufs=1) as big,
            tc.tile_pool(name="binv", bufs=2) as binvp,
            tc.tile_pool(name="zpool", bufs=3) as zpool,
            tc.tile_pool(name="maskp", bufs=1) as maskp,
            tc.tile_pool(name="lhs", bufs=2) as lhs,
            tc.tile_pool(name="small", bufs=1) as small,
            tc.tile_pool(name="ps", bufs=2, space="PSUM") as pspool,
        ):
            # ---- constants ----
            ones_f = small.tile([P, P], F32, name="ones_f")
            nc.vector.memset(ones_f[:], 1.0)
            ones_r = small.tile([P, P], F32R, name="ones_r")
            nc.vector.tensor_copy(ones_r[:], ones_f[:])
            bias_den_t = small.tile([P, 1], F32, name="bias_den_t")
            nc.vector.memset(bias_den_t[:], bias_den)

            # ---- loads (emission order ~ DMA priority) ----
            xtc = []
            ptc = []
            for k in range(KC):
                t = lhs.tile([P, RP], F32, name=f"xtc{k}", tag=f"xtc{k}")
                nc.sync.dma_start(t[:], xTc.ap()[k * P:(k + 1) * P, :])
                xtc.append(t)
                t = lhs.tile([P, RP], F32, name=f"ptc{k}", tag=f"ptc{k}")
                nc.sync.dma_start(t[:], pTc.ap()[k * P:(k + 1) * P, :])
                ptc.append(t)
            xt = []
            for k in range(KC):
                t = xpool.tile([P, N_FULL], F32, name=f"xt{k}", tag="x")
                nc.sync.dma_start(t[:], xT.ap()[k * P:(k + 1) * P, :])
                xt.append(t)
            mask_t = []
            for it in range(IT):
                t = maskp.tile([P, N_FULL], BF16, name=f"mask{it}")
                nc.sync.dma_start(t[:], mk.ap()[it * P:(it + 1) * P, :])
                mask_t.append(t)

            # ---- norms of the core's own x rows and proxy rows (lhsT) ----
            def chunk_norm_inv(src_tiles, label):
                ps_c = pspool.tile([P, JG], F32, name=f"psc_{label}", tag="ps")
                for k in range(KC):
                    sqc = small.tile([P, RP], F32R, name=f"sqc_{label}{k}",
                                     tag="sqc")
                    nc.scalar.activation(sqc[:], src_tiles[k][:], AF.Square)
                    nc.tensor.matmul(ps_c[:, :RP], ones_r[:], sqc[:],
                                     start=(k == 0), stop=(k == KC - 1))
                # 1/sqrt(v) = exp(-0.5 * ln(v)); in-place on the inv tile
                inv = small.tile([P, RP], F32, name=f"inv_{label}")
                nc.scalar.activation(inv[:], ps_c[:, :RP], AF.Ln)
                nc.scalar.activation(inv[:], inv[:], AF.Exp, scale=-0.5)
                return inv

            bx_inv = chunk_norm_inv(xtc, "x")
            bp_inv = chunk_norm_inv(ptc, "p")

            xnc = []
            pnc = []
            for k in range(KC):
                t = lhs.tile([P, RP], F32R, name=f"xnc{k}", tag=f"xtc{k}")
                nc.vector.tensor_tensor(t[:], xtc[k][:], bx_inv[:], ALU.mult)
                xnc.append(t)
                t = lhs.tile([P, RP], F32R, name=f"pnc{k}", tag=f"ptc{k}")
                nc.vector.tensor_tensor(t[:], ptc[k][:], bp_inv[:], ALU.mult)
                pnc.append(t)

            # ---- norms of full x (columns of xT) ----
            ps_norm = [
                pspool.tile([P, JG], F32, name=f"ps_norm{g}", tag="ps")
                for g in range(NG)
            ]
            for k in range(KC):
                sq = big.tile([P, N_FULL], F32R, name=f"sq{k}", tag="bigbuf")
                nc.scalar.activation(sq[:], xt[k][:], AF.Square)
                for g in range(NG):
                    for jl in range(JG // JT):
                        j0 = g * JG + jl * JT
                        nc.tensor.matmul(
                            ps_norm[g][:, jl * JT:(jl + 1) * JT],
                            ones_r[:],
                            sq[:, j0:j0 + JT],
                            start=(k == 0), stop=(k == KC - 1),
                        )
            # 1/||x_j|| broadcast over partitions (per group), via ln/exp
            b_inv = []
            for g in range(NG):
                t = binvp.tile([P, JG], F32, name=f"binv{g}", tag="binv")
                nc.scalar.activation(t[:], ps_norm[g][:], AF.Ln)
                nc.scalar.activation(t[:], t[:], AF.Exp, scale=-0.5)
                b_inv.append(t)

            # normalized xT in fp32r (rhs). Group 0 all on DVE (gates the
            # main-loop start); group 1 split DVE/GPSIMD.
            xn = []
            for k in range(KC):
                t = xpool.tile([P, N_FULL], F32R, name=f"xn{k}", tag="x")
                xn.append(t)
            for k in range(KC):
                nc.vector.tensor_tensor(xn[k][:, :JG], xt[k][:, :JG],
                                        b_inv[0][:], ALU.mult)
            for k in range(KC):
                eng = nc.vector if k < 2 else nc.gpsimd
                eng.tensor_tensor(xn[k][:, JG:], xt[k][:, JG:],
                                  b_inv[1][:], ALU.mult)

            # ---- main loop (group-major for early start) ----
            acc_den = {}
            acc_num = {}
            for it in range(IT):
                for mat in range(2):
                    acc_den[it, mat] = small.tile([P, NG], F32,
                                                  name=f"accd{it}_{mat}")
                    acc_num[it, mat] = small.tile([P, NG], F32,
                                                  name=f"accn{it}_{mat}")
            sums = small.tile([P, 4 * IT], F32, name="sums")
            lns = small.tile([P, 4 * IT], F32, name="lns")

            for g in range(NG):
                for it in range(IT):
                    i0 = it * P
                    for mat in range(2):
                        lh = pnc if mat == 0 else xnc
                        ps = pspool.tile([P, JG], F32,
                                         name=f"ps_{it}_{mat}_{g}", tag="ps")
                        for k in range(KC):
                            for jl in range(JG // JT):
                                j0 = g * JG + jl * JT
                                nc.tensor.matmul(
                                    ps[:, jl * JT:(jl + 1) * JT],
                                    lh[k][:, i0:i0 + P],
                                    xn[k][:, j0:j0 + JT],
                                    start=(k == 0), stop=(k == KC - 1),
                                )
                        z = zpool.tile([P, JG], BF16,
                                       name=f"z_{it}_{mat}_{g}", tag="z")
                        nc.scalar.activation(
                            z[:], ps[:], AF.Exp,
                            bias=bias_den_t[:], scale=inv_t,
                            accum_out=acc_den[it, mat][:, g:g + 1],
                        )
                        zo = zpool.tile([P, JG], BF16,
                                        name=f"zo_{it}_{mat}_{g}", tag="zo",
                                        bufs=2)
                        nc.vector.scalar_tensor_tensor(
                            out=zo[:], in0=z[:], scalar=1.0,
                            in1=mask_t[it][:, g * JG:(g + 1) * JG],
                            op0=ALU.mult, op1=ALU.mult,
                            accum_out=acc_num[it, mat][:, g:g + 1],
                        )
                        # tail per (it, mat) as soon as its last group done
                        if g == NG - 1:
                            cd = it * 4 + mat * 2
                            nc.vector.tensor_reduce(
                                sums[:, cd:cd + 1], acc_den[it, mat][:],
                                AX.X, ALU.add)
                            nc.vector.tensor_reduce(
                                sums[:, cd + 1:cd + 2], acc_num[it, mat][:],
                                AX.X, ALU.add)
                    if g == NG - 1:
                        c0 = it * 4
                        nc.scalar.activation(lns[:, c0:c0 + 4],
                                             sums[:, c0:c0 + 4], AF.Ln)
                        nc.sync.dma_start(out.ap()[it * P:(it + 1) * P, :],
                                          lns[:, c0:c0 + 4])

    nc.compile()
    return nc


def make_in_maps(x, p, nmf, lab):
    xT = np.ascontiguousarray(x.T)
    in_maps = []
    for c in range(N_CORES):
        rows = slice(c * RP, (c + 1) * RP)
        in_maps.append({
            "xT": xT,
            "xTc": np.ascontiguousarray(x[rows].T),
            "pTc": np.ascontiguousarray(p[rows].T),
            "mk": nmf[lab[rows]].astype(ml_dtypes.bfloat16),
        })
    return in_maps


def kernel(inst_embed, proxy, negative_mask, labels, temperature, margin):
    t = float(np.asarray(temperature))
    m = float(np.asarray(margin))
    inv_t = 1.0 / t
    bias_den = -m / t

    key = (t, m)
    if key not in _CACHE:
        _CACHE[key] = _build(inv_t, bias_den)
    nc = _CACHE[key]

    x = np.asarray(inst_embed, dtype=np.float32)
    p = np.asarray(proxy, dtype=np.float32)
    nmf = np.asarray(negative_mask, dtype=np.float32)
    lab = np.asarray(labels).astype(np.int64)

    in_maps = make_in_maps(x, p, nmf, lab)

    res = bass_utils.run_bass_kernel_spmd(nc, in_maps,
                                          core_ids=list(range(N_CORES)))
    outs = np.concatenate([res.results[c]["out"] for c in range(N_CORES)],
                          axis=0)
    ld_p, ln_p, ld_i, ln_i = (outs[:, q].astype(np.float64) for q in range(4))
    loss = (-2.0 * np.log(t)
            + (ld_p - ln_p).mean()
            + (ld_i - ln_i).mean())
    return np.float32(loss)


# revision 9
# speedup vs baseline: 1.5961x; 1.0507x over previous
"""Conditional contrastive loss on 8 TRN2 NeuronCores (Bass/Tile).

Strategy (data-parallel over rows, per sharding hint):
  - Each core owns 512 rows (of 4096) of inst_embed ("x") and proxy ("p").
  - Host passes transposed layouts (matmul-native [D, n]) plus the core's
    pre-gathered positive-selection mask rows (negative_mask[labels]) in bf16.
  - Device normalization: squares on the scalar engine (fp32r out),
    partition-reduction via an all-ones matmul whose M=128 output doubles as
    a partition-broadcast, then 1/sqrt via ACT ln + exp(-0.5*u) (the DVE
    reciprocal is 8 cycles/element - avoid), and DVE/GPSIMD column-scale
    into fp32r.
  - Similarity rows sim[i, j] for the core's i-block: fp32r matmuls
    accumulated in PSUM over 4 K-chunks, 2048 columns per PSUM group.
  - exp((sim-margin)/T) on the scalar engine straight out of PSUM with
    accum_out = row sums -> denominator for free; z written to SBUF in bf16.
  - numerator = scalar_tensor_tensor(z * mask) on DVE with accum_out.
  - Main loop runs group-0 columns first so it can start before group-1
    normalization finishes.
  - Device emits ln(den), ln(num) per row for both matrices ([512, 4] f32
    per core); the host does the final O(N) mean + gather across cores.
"""
import numpy as np
import ml_dtypes

import concourse.bacc as bacc
import concourse.tile as tile
from concourse import mybir, bass_utils

N_FULL = 4096
D = 512
C = 100
N_CORES = 8
RP = N_FULL // N_CORES  # rows per core = 512
P = 128                 # SBUF partitions
KC = D // P             # contraction chunks = 4
JT = 512                # columns per PSUM bank
JG = 2048               # columns per PSUM group (4 banks)
NG = N_FULL // JG       # groups per (i-tile, matrix) = 2
IT = RP // P            # i-tiles per core = 4

F32 = mybir.dt.float32
F32R = mybir.dt.float32r
BF16 = mybir.dt.bfloat16
AF = mybir.ActivationFunctionType
ALU = mybir.AluOpType
AX = mybir.AxisListType

_CACHE = {}


def _build(inv_t: float, bias_den: float):
    nc = bacc.Bacc("TRN2", target_bir_lowering=False, debug=False,
                   num_devices=N_CORES)

    xT = nc.dram_tensor("xT", [D, N_FULL], BF16, kind="ExternalInput")
    xTc = nc.dram_tensor("xTc", [D, RP], BF16, kind="ExternalInput")
    pTc = nc.dram_tensor("pTc", [D, RP], BF16, kind="ExternalInput")
    mk = nc.dram_tensor("mk", [RP, N_FULL], BF16, kind="ExternalInput")
    out = nc.dram_tensor("out", [RP, 4], F32, kind="ExternalOutput")

    with tile.TileContext(nc) as tc:
        with (
            tc.tile_pool(name="xpool", bufs=5) as xpool,
            tc.tile_pool(name="big", bufs=1) as big,
            tc.tile_pool(name="binv", bufs=2) as binvp,
            tc.tile_pool(name="zpool", bufs=3) as zpool,
            tc.tile_pool(name="maskp", bufs=1) as maskp,
            tc.tile_pool(name="lhs", bufs=2) as lhs,
            tc.tile_pool(name="small", bufs=1) as small,
            tc.tile_pool(name="ps", bufs=2, space="PSUM") as pspool,
        ):
            # ---- constants ----
            ones_f = small.tile([P, P], F32, name="ones_f")
            nc.vector.memset(ones_f[:], 1.0)
            ones_r = small.tile([P, P], F32R, name="ones_r")
            nc.vector.tensor_copy(ones_r[:], ones_f[:])
            bias_den_t = small.tile([P, 1], F32, name="bias_den_t")
            nc.vector.memset(bias_den_t[:], bias_den)

            # ---- loads (emission order ~ DMA priority) ----
            xtc = []
            ptc = []
            for k in range(KC):
                t = lhs.tile([P, RP], BF16, name=f"xtc{k}", tag=f"xtc{k}")
                nc.sync.dma_start(t[:], xTc.ap()[k * P:(k + 1) * P, :])
                xtc.append(t)
                t = lhs.tile([P, RP], BF16, name=f"ptc{k}", tag=f"ptc{k}")
                nc.sync.dma_start(t[:], pTc.ap()[k * P:(k + 1) * P, :])
                ptc.append(t)
            xt = []
            for k in range(KC):
                t = xpool.tile([P, N_FULL], BF16, name=f"xt{k}", tag="x")
                nc.sync.dma_start(t[:], xT.ap()[k * P:(k + 1) * P, :])
                xt.append(t)
            mask_t = []
            for it in range(IT):
                t = maskp.tile([P, N_FULL], BF16, name=f"mask{it}")
                nc.sync.dma_start(t[:], mk.ap()[it * P:(it + 1) * P, :])
                mask_t.append(t)

            # ---- norms of the core's own x rows and proxy rows (lhsT) ----
            def chunk_norm_inv(src_tiles, label):
                ps_c = pspool.tile([P, JG], F32, name=f"psc_{label}", tag="ps")
                for k in range(KC):
                    sqc = small.tile([P, RP], F32R, name=f"sqc_{label}{k}",
                                     tag="sqc")
                    nc.scalar.activation(sqc[:], src_tiles[k][:], AF.Square)
                    nc.tensor.matmul(ps_c[:, :RP], ones_r[:], sqc[:],
                                     start=(k == 0), stop=(k == KC - 1))
                # 1/sqrt(v) = exp(-0.5 * ln(v)); in-place on the inv tile
                inv = small.tile([P, RP], F32, name=f"inv_{label}")
                nc.scalar.activation(inv[:], ps_c[:, :RP], AF.Ln)
                nc.scalar.activation(inv[:], inv[:], AF.Exp, scale=-0.5)
                return inv

            bx_inv = chunk_norm_inv(xtc, "x")
            bp_inv = chunk_norm_inv(ptc, "p")

            xnc = []
            pnc = []
            for k in range(KC):
                t = lhs.tile([P, RP], BF16, name=f"xnc{k}", tag=f"xnc{k}")
                nc.vector.tensor_tensor(t[:], xtc[k][:], bx_inv[:], ALU.mult)
                xnc.append(t)
                t = lhs.tile([P, RP], BF16, name=f"pnc{k}", tag=f"pnc{k}")
                nc.vector.tensor_tensor(t[:], ptc[k][:], bp_inv[:], ALU.mult)
                pnc.append(t)

            # ---- norms of full x (columns of xT) ----
            ps_norm = [
                pspool.tile([P, JG], F32, name=f"ps_norm{g}", tag="ps")
                for g in range(NG)
            ]
            for k in range(KC):
                sq = big.tile([P, N_FULL], F32R, name=f"sq{k}", tag="bigbuf")
                nc.scalar.activation(sq[:], xt[k][:], AF.Square)
                for g in range(NG):
                    for jl in range(JG // JT):
                        j0 = g * JG + jl * JT
                        nc.tensor.matmul(
                            ps_norm[g][:, jl * JT:(jl + 1) * JT],
                            ones_r[:],
                            sq[:, j0:j0 + JT],
                            start=(k == 0), stop=(k == KC - 1),
                        )
            # 1/||x_j|| broadcast over partitions (per group), via ln/exp
            b_inv = []
            for g in range(NG):
                t = binvp.tile([P, JG], F32, name=f"binv{g}", tag="binv")
                nc.scalar.activation(t[:], ps_norm[g][:], AF.Ln)
                nc.scalar.activation(t[:], t[:], AF.Exp, scale=-0.5)
                b_inv.append(t)

            # normalized xT in fp32r (rhs). Group 0 all on DVE (gates the
            # main-loop start); group 1 split DVE/GPSIMD.
            xn = []
            for k in range(KC):
                t = xpool.tile([P, N_FULL], BF16, name=f"xn{k}", tag="x")
                xn.append(t)
            for k in range(KC):
                nc.vector.tensor_tensor(xn[k][:, :JG], xt[k][:, :JG],
                                        b_inv[0][:], ALU.mult)
            for k in range(KC):
                eng = nc.vector if k < 2 else nc.gpsimd
                eng.tensor_tensor(xn[k][:, JG:], xt[k][:, JG:],
                                  b_inv[1][:], ALU.mult)

            # ---- main loop (group-major for early start) ----
            acc_den = {}
            acc_num = {}
            for it in range(IT):
                for mat in range(2):
                    acc_den[it, mat] = small.tile([P, NG], F32,
                                                  name=f"accd{it}_{mat}")
                    acc_num[it, mat] = small.tile([P, NG], F32,
                                                  name=f"accn{it}_{mat}")
            sums = small.tile([P, 4 * IT], F32, name="sums")
            lns = small.tile([P, 4 * IT], F32, name="lns")

            for g in range(NG):
                for it in range(IT):
                    i0 = it * P
                    for mat in range(2):
                        lh = pnc if mat == 0 else xnc
                        ps = pspool.tile([P, JG], F32,
                                         name=f"ps_{it}_{mat}_{g}", tag="ps")
                        for k in range(KC):
                            for jl in range(JG // JT):
                                j0 = g * JG + jl * JT
                                nc.tensor.matmul(
                                    ps[:, jl * JT:(jl + 1) * JT],
                                    lh[k][:, i0:i0 + P],
                                    xn[k][:, j0:j0 + JT],
                                    start=(k == 0), stop=(k == KC - 1),
                                )
                        z = zpool.tile([P, JG], BF16,
                                       name=f"z_{it}_{mat}_{g}", tag="z")
                        nc.scalar.activation(
                            z[:], ps[:], AF.Exp,
                            bias=bias_den_t[:], scale=inv_t,
                            accum_out=acc_den[it, mat][:, g:g + 1],
                        )
                        zo = zpool.tile([P, JG], BF16,
                                        name=f"zo_{it}_{mat}_{g}", tag="zo",
                                        bufs=2)
                        nc.vector.scalar_tensor_tensor(
                            out=zo[:], in0=z[:], scalar=1.0,
                            in1=mask_t[it][:, g * JG:(g + 1) * JG],
                            op0=ALU.mult, op1=ALU.mult,
                            accum_out=acc_num[it, mat][:, g:g + 1],
                        )
                        # tail per (it, mat) as soon as its last group done
                        if g == NG - 1:
                            cd = it * 4 + mat * 2
                            nc.vector.tensor_reduce(
                                sums[:, cd:cd + 1], acc_den[it, mat][:],
                                AX.X, ALU.add)
                            nc.vector.tensor_reduce(
                                sums[:, cd + 1:cd + 2], acc_num[it, mat][:],
                                AX.X, ALU.add)
                    if g == NG - 1:
                        c0 = it * 4
                        nc.scalar.activation(lns[:, c0:c0 + 4],
                                             sums[:, c0:c0 + 4], AF.Ln)
                        nc.sync.dma_start(out.ap()[it * P:(it + 1) * P, :],
                                          lns[:, c0:c0 + 4])

    nc.compile()
    return nc


def make_in_maps(x, p, nmf, lab):
    xT = np.ascontiguousarray(x.T.astype(ml_dtypes.bfloat16))
    in_maps = []
    for c in range(N_CORES):
        rows = slice(c * RP, (c + 1) * RP)
        in_maps.append({
            "xT": xT,
            "xTc": np.ascontiguousarray(xT[:, rows]),
            "pTc": np.ascontiguousarray(p[rows].T.astype(ml_dtypes.bfloat16)),
            "mk": nmf[lab[rows]].astype(ml_dtypes.bfloat16),
        })
    return in_maps


def kernel(inst_embed, proxy, negative_mask, labels, temperature, margin):
    t = float(np.asarray(temperature))
    m = float(np.asarray(margin))
    inv_t = 1.0 / t
    bias_den = -m / t

    key = (t, m)
    if key not in _CACHE:
        _CACHE[key] = _build(inv_t, bias_den)
    nc = _CACHE[key]

    x = np.asarray(inst_embed, dtype=np.float32)
    p = np.asarray(proxy, dtype=np.float32)
    nmf = np.asarray(negative_mask, dtype=np.float32)
    lab = np.asarray(labels).astype(np.int64)

    in_maps = make_in_maps(x, p, nmf, lab)

    res = bass_utils.run_bass_kernel_spmd(nc, in_maps,
                                          core_ids=list(range(N_CORES)))
    outs = np.concatenate([res.results[c]["out"] for c in range(N_CORES)],
                          axis=0)
    ld_p, ln_p, ld_i, ln_i = (outs[:, q].astype(np.float64) for q in range(4))
    loss = (-2.0 * np.log(t)
            + (ld_p - ln_p).mean()
            + (ld_i - ln_i).mean())
    return np.float32(loss)


# revision 10
# speedup vs baseline: 1.6246x; 1.0179x over previous
"""Conditional contrastive loss on 8 TRN2 NeuronCores (Bass/Tile).

Strategy (data-parallel over rows, per sharding hint):
  - Each core owns 512 rows (of 4096) of inst_embed ("x") and proxy ("p").
  - Host passes transposed layouts (matmul-native [D, n]) plus the core's
    pre-gathered positive-selection mask rows (negative_mask[labels]) in bf16.
  - Device normalization: squares on the scalar engine (fp32r out),
    partition-reduction via an all-ones matmul whose M=128 output doubles as
    a partition-broadcast, then 1/sqrt via ACT ln + exp(-0.5*u) (the DVE
    reciprocal is 8 cycles/element - avoid), and DVE/GPSIMD column-scale
    into fp32r.
  - Similarity rows sim[i, j] for the core's i-block: fp32r matmuls
    accumulated in PSUM over 4 K-chunks, 2048 columns per PSUM group.
  - exp((sim-margin)/T) on the scalar engine straight out of PSUM with
    accum_out = row sums -> denominator for free; z written to SBUF in bf16.
  - numerator = scalar_tensor_tensor(z * mask) on DVE with accum_out.
  - Main loop runs group-0 columns first so it can start before group-1
    normalization finishes.
  - Device emits ln(den), ln(num) per row for both matrices ([512, 4] f32
    per core); the host does the final O(N) mean + gather across cores.
"""
import numpy as np
import ml_dtypes

import concourse.bacc as bacc
import concourse.tile as tile
from concourse import mybir, bass_utils

N_FULL = 4096
D = 512
C = 100
N_CORES = 8
RP = N_FULL // N_CORES  # rows per core = 512
P = 128                 # SBUF partitions
KC = D // P             # contraction chunks = 4
JT = 512                # columns per PSUM bank
JG = 2048               # columns per PSUM group (4 banks)
NG = N_FULL // JG       # groups per (i-tile, matrix) = 2
IT = RP // P            # i-tiles per core = 4

F32 = mybir.dt.float32
F32R = mybir.dt.float32r
BF16 = mybir.dt.bfloat16
AF = mybir.ActivationFunctionType
ALU = mybir.AluOpType
AX = mybir.AxisListType

_CACHE = {}


def _build(inv_t: float, bias_den: float):
    nc = bacc.Bacc("TRN2", target_bir_lowering=False, debug=False,
                   num_devices=N_CORES)

    xT = nc.dram_tensor("xT", [D, N_FULL], BF16, kind="ExternalInput")
    xTc = nc.dram_tensor("xTc", [D, RP], BF16, kind="ExternalInput")
    pTc = nc.dram_tensor("pTc", [D, RP], BF16, kind="ExternalInput")
    mk = nc.dram_tensor("mk", [RP, N_FULL], BF16, kind="ExternalInput")
    out = nc.dram_tensor("out", [RP, 4], F32, kind="ExternalOutput")

    with tile.TileContext(nc) as tc:
        with (
            tc.tile_pool(name="xpool", bufs=5) as xpool,
            tc.tile_pool(name="big", bufs=1) as big,
            tc.tile_pool(name="binv", bufs=2) as binvp,
            tc.tile_pool(name="zpool", bufs=3) as zpool,
            tc.tile_pool(name="maskp", bufs=1) as maskp,
            tc.tile_pool(name="lhs", bufs=2) as lhs,
            tc.tile_pool(name="small", bufs=1) as small,
            tc.tile_pool(name="ps", bufs=2, space="PSUM") as pspool,
        ):
            # ---- constants ----
            ones_r = small.tile([P, P], BF16, name="ones_r")
            nc.vector.memset(ones_r[:], 1.0)
            bias_den_t = small.tile([P, 1], F32, name="bias_den_t")
            nc.vector.memset(bias_den_t[:], bias_den)

            # ---- loads (emission order ~ DMA priority) ----
            xtc = []
            ptc = []
            for k in range(KC):
                t = lhs.tile([P, RP], BF16, name=f"xtc{k}", tag=f"xtc{k}")
                nc.sync.dma_start(t[:], xTc.ap()[k * P:(k + 1) * P, :])
                xtc.append(t)
                t = lhs.tile([P, RP], BF16, name=f"ptc{k}", tag=f"ptc{k}")
                nc.sync.dma_start(t[:], pTc.ap()[k * P:(k + 1) * P, :])
                ptc.append(t)
            xt = []
            for k in range(KC):
                t = xpool.tile([P, N_FULL], BF16, name=f"xt{k}", tag="x")
                nc.sync.dma_start(t[:], xT.ap()[k * P:(k + 1) * P, :])
                xt.append(t)
            mask_t = []
            for it in range(IT):
                t = maskp.tile([P, N_FULL], BF16, name=f"mask{it}")
                nc.sync.dma_start(t[:], mk.ap()[it * P:(it + 1) * P, :])
                mask_t.append(t)

            # ---- norms of the core's own x rows and proxy rows (lhsT) ----
            def chunk_norm_inv(src_tiles, label):
                ps_c = pspool.tile([P, JG], F32, name=f"psc_{label}", tag="ps")
                for k in range(KC):
                    sqc = small.tile([P, RP], BF16, name=f"sqc_{label}{k}",
                                     tag="sqc")
                    nc.vector.tensor_tensor(sqc[:], src_tiles[k][:],
                                            src_tiles[k][:], ALU.mult)
                    nc.tensor.matmul(ps_c[:, :RP], ones_r[:], sqc[:],
                                     start=(k == 0), stop=(k == KC - 1))
                # 1/sqrt(v) = exp(-0.5 * ln(v)); in-place on the inv tile
                inv = small.tile([P, RP], F32, name=f"inv_{label}")
                nc.scalar.activation(inv[:], ps_c[:, :RP], AF.Ln)
                nc.scalar.activation(inv[:], inv[:], AF.Exp, scale=-0.5)
                return inv

            bx_inv = chunk_norm_inv(xtc, "x")
            bp_inv = chunk_norm_inv(ptc, "p")

            xnc = []
            pnc = []
            for k in range(KC):
                t = lhs.tile([P, RP], BF16, name=f"xnc{k}", tag=f"xnc{k}")
                nc.vector.tensor_tensor(t[:], xtc[k][:], bx_inv[:], ALU.mult)
                xnc.append(t)
                t = lhs.tile([P, RP], BF16, name=f"pnc{k}", tag=f"pnc{k}")
                nc.vector.tensor_tensor(t[:], ptc[k][:], bp_inv[:], ALU.mult)
                pnc.append(t)

            # ---- norms of full x (columns of xT) ----
            ps_norm = [
                pspool.tile([P, JG], F32, name=f"ps_norm{g}", tag="ps")
                for g in range(NG)
            ]
            for k in range(KC):
                sq = big.tile([P, N_FULL], BF16, name=f"sq{k}", tag="bigbuf",
                              bufs=2)
                eng = nc.vector if k < 2 else nc.gpsimd
                eng.tensor_tensor(sq[:], xt[k][:], xt[k][:], ALU.mult)
                for g in range(NG):
                    for jl in range(JG // JT):
                        j0 = g * JG + jl * JT
                        nc.tensor.matmul(
                            ps_norm[g][:, jl * JT:(jl + 1) * JT],
                            ones_r[:],
                            sq[:, j0:j0 + JT],
                            start=(k == 0), stop=(k == KC - 1),
                        )
            # 1/||x_j|| broadcast over partitions (per group), via ln/exp
            b_inv = []
            for g in range(NG):
                t = binvp.tile([P, JG], F32, name=f"binv{g}", tag="binv")
                nc.scalar.activation(t[:], ps_norm[g][:], AF.Ln)
                nc.scalar.activation(t[:], t[:], AF.Exp, scale=-0.5)
                b_inv.append(t)

            # normalized xT in fp32r (rhs). Group 0 all on DVE (gates the
            # main-loop start); group 1 split DVE/GPSIMD.
            xn = []
            for k in range(KC):
                t = xpool.tile([P, N_FULL], BF16, name=f"xn{k}", tag="x")
                xn.append(t)
            for k in range(KC):
                nc.vector.tensor_tensor(xn[k][:, :JG], xt[k][:, :JG],
                                        b_inv[0][:], ALU.mult)
            for k in range(KC):
                eng = nc.vector if k < 1 else nc.gpsimd
                eng.tensor_tensor(xn[k][:, JG:], xt[k][:, JG:],
                                  b_inv[1][:], ALU.mult)

            # ---- main loop (group-major for early start) ----
            acc_den = {}
            acc_num = {}
            for it in range(IT):
                for mat in range(2):
                    acc_den[it, mat] = small.tile([P, NG], F32,
                                                  name=f"accd{it}_{mat}")
                    acc_num[it, mat] = small.tile([P, NG], F32,
                                                  name=f"accn{it}_{mat}")
            sums = small.tile([P, 4 * IT], F32, name="sums")
            lns = small.tile([P, 4 * IT], F32, name="lns")

            for g in range(NG):
                for it in range(IT):
                    i0 = it * P
                    for mat in range(2):
                        lh = pnc if mat == 0 else xnc
                        ps = pspool.tile([P, JG], F32,
                                         name=f"ps_{it}_{mat}_{g}", tag="ps")
                        for k in range(KC):
                            for jl in range(JG // JT):
                                j0 = g * JG + jl * JT
                                nc.tensor.matmul(
                                    ps[:, jl * JT:(jl + 1) * JT],
                                    lh[k][:, i0:i0 + P],
                                    xn[k][:, j0:j0 + JT],
                                    start=(k == 0), stop=(k == KC - 1),
                                )
                        z = zpool.tile([P, JG], BF16,
                                       name=f"z_{it}_{mat}_{g}", tag="z")
                        nc.scalar.activation(
                            z[:], ps[:], AF.Exp,
                            bias=bias_den_t[:], scale=inv_t,
                            accum_out=acc_den[it, mat][:, g:g + 1],
                        )
                        zo = zpool.tile([P, JG], BF16,
                                        name=f"zo_{it}_{mat}_{g}", tag="zo",
                                        bufs=2)
                        nc.vector.scalar_tensor_tensor(
                            out=zo[:], in0=z[:], scalar=1.0,
                            in1=mask_t[it][:, g * JG:(g + 1) * JG],
                            op0=ALU.mult, op1=ALU.mult,
                            accum_out=acc_num[it, mat][:, g:g + 1],
                        )
                        # tail per (it, mat) as soon as its last group done
                        if g == NG - 1:
                            cd = it * 4 + mat * 2
                            nc.vector.tensor_reduce(
                                sums[:, cd:cd + 1], acc_den[it, mat][:],
                                AX.X, ALU.add)
                            nc.vector.tensor_reduce(
                                sums[:, cd + 1:cd + 2], acc_num[it, mat][:],
                                AX.X, ALU.add)
                    if g == NG - 1:
                        c0 = it * 4
                        nc.scalar.activation(lns[:, c0:c0 + 4],
                                             sums[:, c0:c0 + 4], AF.Ln)
                        nc.sync.dma_start(out.ap()[it * P:(it + 1) * P, :],
                                          lns[:, c0:c0 + 4])

    nc.compile()
    return nc


def make_in_maps(x, p, nmf, lab):
    xT = np.ascontiguousarray(x.T.astype(ml_dtypes.bfloat16))
    in_maps = []
    for c in range(N_CORES):
        rows = slice(c * RP, (c + 1) * RP)
        in_maps.append({
            "xT": xT,
            "xTc": np.ascontiguousarray(xT[:, rows]),
            "pTc": np.ascontiguousarray(p[rows].T.astype(ml_dtypes.bfloat16)),
            "mk": nmf[lab[rows]].astype(ml_dtypes.bfloat16),
        })
    return in_maps


def kernel(inst_embed, proxy, negative_mask, labels, temperature, margin):
    t = float(np.asarray(temperature))
    m = float(np.asarray(margin))
    inv_t = 1.0 / t
    bias_den = -m / t

    key = (t, m)
    if key not in _CACHE:
        _CACHE[key] = _build(inv_t, bias_den)
    nc = _CACHE[key]

    x = np.asarray(inst_embed, dtype=np.float32)
    p = np.asarray(proxy, dtype=np.float32)
    nmf = np.asarray(negative_mask, dtype=np.float32)
    lab = np.asarray(labels).astype(np.int64)

    in_maps = make_in_maps(x, p, nmf, lab)

    res = bass_utils.run_bass_kernel_spmd(nc, in_maps,
                                          core_ids=list(range(N_CORES)))
    outs = np.concatenate([res.results[c]["out"] for c in range(N_CORES)],
                          axis=0)
    ld_p, ln_p, ld_i, ln_i = (outs[:, q].astype(np.float64) for q in range(4))
    loss = (-2.0 * np.log(t)
            + (ld_p - ln_p).mean()
            + (ld_i - ln_i).mean())
    return np.float32(loss)


# revision 11
# speedup vs baseline: 1.7532x; 1.0792x over previous
"""Conditional contrastive loss on 8 TRN2 NeuronCores (Bass/Tile).

Strategy (data-parallel over rows, per sharding hint):
  - Each core owns 512 rows (of 4096) of inst_embed ("x") and proxy ("p").
  - Host passes transposed layouts (matmul-native [D, n]) plus the core's
    pre-gathered positive-selection mask rows (negative_mask[labels]) in bf16.
  - Device normalization: squares on the scalar engine (fp32r out),
    partition-reduction via an all-ones matmul whose M=128 output doubles as
    a partition-broadcast, then 1/sqrt via ACT ln + exp(-0.5*u) (the DVE
    reciprocal is 8 cycles/element - avoid), and DVE/GPSIMD column-scale
    into fp32r.
  - Similarity rows sim[i, j] for the core's i-block: fp32r matmuls
    accumulated in PSUM over 4 K-chunks, 2048 columns per PSUM group.
  - exp((sim-margin)/T) on the scalar engine straight out of PSUM with
    accum_out = row sums -> denominator for free; z written to SBUF in bf16.
  - numerator = scalar_tensor_tensor(z * mask) on DVE with accum_out.
  - Main loop runs group-0 columns first so it can start before group-1
    normalization finishes.
  - Device emits ln(den), ln(num) per row for both matrices ([512, 4] f32
    per core); the host does the final O(N) mean + gather across cores.
"""
import numpy as np
import ml_dtypes

import concourse.bacc as bacc
import concourse.tile as tile
from concourse import mybir, bass_utils

N_FULL = 4096
D = 512
C = 100
N_CORES = 8
RP = N_FULL // N_CORES  # rows per core = 512
P = 128                 # SBUF partitions
KC = D // P             # contraction chunks = 4
JT = 512                # columns per PSUM bank
JG = 2048               # columns per PSUM group (4 banks)
NG = N_FULL // JG       # groups per (i-tile, matrix) = 2
IT = RP // P            # i-tiles per core = 4

F32 = mybir.dt.float32
F32R = mybir.dt.float32r
BF16 = mybir.dt.bfloat16
AF = mybir.ActivationFunctionType
ALU = mybir.AluOpType
AX = mybir.AxisListType

_CACHE = {}


def _build(inv_t: float, bias_den: float):
    nc = bacc.Bacc("TRN2", target_bir_lowering=False, debug=False,
                   num_devices=N_CORES)

    xT = nc.dram_tensor("xT", [D, N_FULL], BF16, kind="ExternalInput")
    xTc = nc.dram_tensor("xTc", [D, RP], BF16, kind="ExternalInput")
    pTc = nc.dram_tensor("pTc", [D, RP], BF16, kind="ExternalInput")
    mk = nc.dram_tensor("mk", [RP, N_FULL], BF16, kind="ExternalInput")
    out = nc.dram_tensor("out", [RP, 4], F32, kind="ExternalOutput")

    with tile.TileContext(nc) as tc:
        with (
            tc.tile_pool(name="xpool", bufs=5) as xpool,
            tc.tile_pool(name="big", bufs=1) as big,
            tc.tile_pool(name="binv", bufs=2) as binvp,
            tc.tile_pool(name="zpool", bufs=3) as zpool,
            tc.tile_pool(name="maskp", bufs=1) as maskp,
            tc.tile_pool(name="lhs", bufs=2) as lhs,
            tc.tile_pool(name="small", bufs=1) as small,
            tc.tile_pool(name="ps", bufs=2, space="PSUM") as pspool,
        ):
            # ---- constants ----
            ones_r = small.tile([P, P], BF16, name="ones_r")
            nc.vector.memset(ones_r[:], 1.0)
            bias_den_t = small.tile([P, 1], F32, name="bias_den_t")
            nc.vector.memset(bias_den_t[:], bias_den)

            # ---- loads (emission order ~ DMA priority) ----
            xtc = []
            ptc = []
            for k in range(KC):
                t = lhs.tile([P, RP], BF16, name=f"xtc{k}", tag=f"xtc{k}")
                nc.sync.dma_start(t[:], xTc.ap()[k * P:(k + 1) * P, :])
                xtc.append(t)
                t = lhs.tile([P, RP], BF16, name=f"ptc{k}", tag=f"ptc{k}")
                nc.sync.dma_start(t[:], pTc.ap()[k * P:(k + 1) * P, :])
                ptc.append(t)
            xt = []
            for k in range(KC):
                t = xpool.tile([P, N_FULL], BF16, name=f"xt{k}", tag="x")
                nc.sync.dma_start(t[:], xT.ap()[k * P:(k + 1) * P, :])
                xt.append(t)
            mask_t = []
            for it in range(IT):
                t = maskp.tile([P, N_FULL], BF16, name=f"mask{it}")
                nc.sync.dma_start(t[:], mk.ap()[it * P:(it + 1) * P, :])
                mask_t.append(t)

            # ---- norms of the core's own x rows and proxy rows (lhsT) ----
            def chunk_norm_inv(src_tiles, label):
                ps_c = pspool.tile([P, JG], F32, name=f"psc_{label}", tag="ps")
                for k in range(KC):
                    sqc = small.tile([P, RP], BF16, name=f"sqc_{label}{k}",
                                     tag="sqc")
                    nc.vector.tensor_tensor(sqc[:], src_tiles[k][:],
                                            src_tiles[k][:], ALU.mult)
                    nc.tensor.matmul(ps_c[:, :RP], ones_r[:], sqc[:],
                                     start=(k == 0), stop=(k == KC - 1))
                # 1/sqrt(v) = exp(-0.5 * ln(v)); in-place on the inv tile
                inv = small.tile([P, RP], F32, name=f"inv_{label}")
                nc.scalar.activation(inv[:], ps_c[:, :RP], AF.Ln)
                nc.scalar.activation(inv[:], inv[:], AF.Exp, scale=-0.5)
                return inv

            bx_inv = chunk_norm_inv(xtc, "x")
            bp_inv = chunk_norm_inv(ptc, "p")

            xnc = []
            pnc = []
            for k in range(KC):
                t = lhs.tile([P, RP], BF16, name=f"xnc{k}", tag=f"xnc{k}")
                nc.vector.tensor_tensor(t[:], xtc[k][:], bx_inv[:], ALU.mult)
                xnc.append(t)
                t = lhs.tile([P, RP], BF16, name=f"pnc{k}", tag=f"pnc{k}")
                nc.vector.tensor_tensor(t[:], ptc[k][:], bp_inv[:], ALU.mult)
                pnc.append(t)

            # ---- norms of full x (columns of xT) ----
            ps_norm = [
                pspool.tile([P, JG], F32, name=f"ps_norm{g}", tag="ps")
                for g in range(NG)
            ]
            for k in range(KC):
                sq = big.tile([P, N_FULL], BF16, name=f"sq{k}", tag="bigbuf",
                              bufs=2)
                nc.vector.tensor_tensor(sq[:], xt[k][:], xt[k][:], ALU.mult)
                for g in range(NG):
                    for jl in range(JG // JT):
                        j0 = g * JG + jl * JT
                        nc.tensor.matmul(
                            ps_norm[g][:, jl * JT:(jl + 1) * JT],
                            ones_r[:],
                            sq[:, j0:j0 + JT],
                            start=(k == 0), stop=(k == KC - 1),
                        )
            # 1/||x_j|| broadcast over partitions (per group), via ln/exp
            b_inv = []
            for g in range(NG):
                t = binvp.tile([P, JG], F32, name=f"binv{g}", tag="binv")
                nc.scalar.activation(t[:], ps_norm[g][:], AF.Ln)
                nc.scalar.activation(t[:], t[:], AF.Exp, scale=-0.5)
                b_inv.append(t)

            # normalized xT in fp32r (rhs). Group 0 all on DVE (gates the
            # main-loop start); group 1 split DVE/GPSIMD.
            xn = []
            for k in range(KC):
                t = xpool.tile([P, N_FULL], BF16, name=f"xn{k}", tag="x")
                xn.append(t)
            for k in range(KC):
                nc.vector.tensor_tensor(xn[k][:, :JG], xt[k][:, :JG],
                                        b_inv[0][:], ALU.mult)
            for k in range(KC):
                eng = nc.vector if k < 1 else nc.gpsimd
                eng.tensor_tensor(xn[k][:, JG:], xt[k][:, JG:],
                                  b_inv[1][:], ALU.mult)

            # ---- main loop (group-major for early start) ----
            acc_den = {}
            acc_num = {}
            for it in range(IT):
                for mat in range(2):
                    acc_den[it, mat] = small.tile([P, NG], F32,
                                                  name=f"accd{it}_{mat}")
                    acc_num[it, mat] = small.tile([P, NG], F32,
                                                  name=f"accn{it}_{mat}")
            sums = small.tile([P, 4 * IT], F32, name="sums")
            lns = small.tile([P, 4 * IT], F32, name="lns")

            for g in range(NG):
                for it in range(IT):
                    i0 = it * P
                    for mat in range(2):
                        lh = pnc if mat == 0 else xnc
                        ps = pspool.tile([P, JG], F32,
                                         name=f"ps_{it}_{mat}_{g}", tag="ps")
                        for k in range(KC):
                            for jl in range(JG // JT):
                                j0 = g * JG + jl * JT
                                nc.tensor.matmul(
                                    ps[:, jl * JT:(jl + 1) * JT],
                                    lh[k][:, i0:i0 + P],
                                    xn[k][:, j0:j0 + JT],
                                    start=(k == 0), stop=(k == KC - 1),
                                )
                        z = zpool.tile([P, JG], BF16,
                                       name=f"z_{it}_{mat}_{g}", tag="z")
                        nc.scalar.activation(
                            z[:], ps[:], AF.Exp,
                            bias=bias_den_t[:], scale=inv_t,
                            accum_out=acc_den[it, mat][:, g:g + 1],
                        )
                        zo = zpool.tile([P, JG], BF16,
                                        name=f"zo_{it}_{mat}_{g}", tag="zo",
                                        bufs=2)
                        nc.vector.scalar_tensor_tensor(
                            out=zo[:], in0=z[:], scalar=1.0,
                            in1=mask_t[it][:, g * JG:(g + 1) * JG],
                            op0=ALU.mult, op1=ALU.mult,
                            accum_out=acc_num[it, mat][:, g:g + 1],
                        )
                        # tail per (it, mat) as soon as its last group done
                        if g == NG - 1:
                            cd = it * 4 + mat * 2
                            nc.vector.tensor_reduce(
                                sums[:, cd:cd + 1], acc_den[it, mat][:],
                                AX.X, ALU.add)
                            nc.vector.tensor_reduce(
                                sums[:, cd + 1:cd + 2], acc_num[it, mat][:],
                                AX.X, ALU.add)
                    if g == NG - 1:
                        c0 = it * 4
                        nc.scalar.activation(lns[:, c0:c0 + 4],
                                             sums[:, c0:c0 + 4], AF.Ln)
                        nc.sync.dma_start(out.ap()[it * P:(it + 1) * P, :],
                                          lns[:, c0:c0 + 4])

    nc.compile()
    return nc


def make_in_maps(x, p, nmf, lab):
    xT = np.ascontiguousarray(x.T.astype(ml_dtypes.bfloat16))
    in_maps = []
    for c in range(N_CORES):
        rows = slice(c * RP, (c + 1) * RP)
        in_maps.append({
            "xT": xT,
            "xTc": np.ascontiguousarray(xT[:, rows]),
            "pTc": np.ascontiguousarray(p[rows].T.astype(ml_dtypes.bfloat16)),
            "mk": nmf[lab[rows]].astype(ml_dtypes.bfloat16),
        })
    return in_maps


def kernel(inst_embed, proxy, negative_mask, labels, temperature, margin):
    t = float(np.asarray(temperature))
    m = float(np.asarray(margin))
    inv_t = 1.0 / t
    bias_den = -m / t

    key = (t, m)
    if key not in _CACHE:
        _CACHE[key] = _build(inv_t, bias_den)
    nc = _CACHE[key]

    x = np.asarray(inst_embed, dtype=np.float32)
    p = np.asarray(proxy, dtype=np.float32)
    nmf = np.asarray(negative_mask, dtype=np.float32)
    lab = np.asarray(labels).astype(np.int64)

    in_maps = make_in_maps(x, p, nmf, lab)

    res = bass_utils.run_bass_kernel_spmd(nc, in_maps,
                                          core_ids=list(range(N_CORES)))
    outs = np.concatenate([res.results[c]["out"] for c in range(N_CORES)],
                          axis=0)
    ld_p, ln_p, ld_i, ln_i = (outs[:, q].astype(np.float64) for q in range(4))
    loss = (-2.0 * np.log(t)
            + (ld_p - ln_p).mean()
            + (ld_i - ln_i).mean())
    return np.float32(loss)
